# revision 14
# baseline (speedup 1.0000x reference)
"""Trainium2 Bass kernel for nn_LocalEncoder (masked GRU + attention pooling).

v2 strategy (data-parallel over batch, 8 cores x 512 rows, 2 chunks of 256):
- Feature-major layout [U partitions, batch free]; bf16 matmuls -> fp32 PSUM.
- Scan: x-projections for BOTH chunks fused 512-wide (chunks side by side in
  the free dim); recurrent matmuls 256-wide per chunk accumulate into halves.
  No bias matmuls: z/r/h-recurrent biases ride a constant-ones row (row 100)
  of the h tile; the candidate x-bias rides scalar_tensor_tensor's
  per-partition scalar. Trailing-padding mask folded via xaug row 100
  (1-m, scaled -40 in the z weights) freezing h exactly like the reference.
- DMAs batched over G=8 timesteps (xaug in, state out, state in).
- d = hh - h runs on GpSimd to unload the Vector engine.
- Attention: 512-wide timestep pairs; A1@[last|last] accumulated in PSUM;
  alpha*state accumulated via identity-matmul into a 512-wide PSUM bank,
  folded once at the end. Device uses last@A1 for ALL t; host subtracts the
  closed-form correction for masked (trailing) timesteps.
"""
import sys
sys.path.insert(0, "/opt/trn_rl_repo")
from contextlib import ExitStack

import numpy as np
import ml_dtypes

import concourse.bass as bass
import concourse.bacc as bacc
import concourse.tile as tile
from concourse import mybir
from concourse import bass_utils

bf16 = ml_dtypes.bfloat16
AF = mybir.ActivationFunctionType
OP = mybir.AluOpType

B, T, E, U = 4096, 200, 100, 100
NCORES = 8
BC = 256          # chunk width
NCHUNK = 2
PERCORE = BC * NCHUNK
G = 8             # timesteps per DMA block
NB = T // G

_CACHE = {}


def _build():
    nc = bacc.Bacc()
    dt = mybir.dt
    xaug = nc.dram_tensor("xaug", [NB, 101, G * 512], dt.bfloat16, kind="ExternalInput")
    wKzN = nc.dram_tensor("wKzN", [101, U], dt.bfloat16, kind="ExternalInput")
    wKr = nc.dram_tensor("wKr", [101, U], dt.bfloat16, kind="ExternalInput")
    wKh = nc.dram_tensor("wKh", [101, U], dt.bfloat16, kind="ExternalInput")
    wRzN = nc.dram_tensor("wRzN", [101, U], dt.bfloat16, kind="ExternalInput")
    wRr = nc.dram_tensor("wRr", [101, U], dt.bfloat16, kind="ExternalInput")
    wRh = nc.dram_tensor("wRh", [101, U], dt.bfloat16, kind="ExternalInput")
    wb0h = nc.dram_tensor("wb0h", [U, 1], dt.float32, kind="ExternalInput")
    wones = nc.dram_tensor("wones", [1, G * BC], dt.bfloat16, kind="ExternalInput")
    wA1 = nc.dram_tensor("wA1", [U, U], dt.bfloat16, kind="ExternalInput")
    wA2 = nc.dram_tensor("wA2", [U, U], dt.bfloat16, kind="ExternalInput")
    wVr = nc.dram_tensor("wVr", [U, U], dt.bfloat16, kind="ExternalInput")
    wI = nc.dram_tensor("wI", [U, U], dt.bfloat16, kind="ExternalInput")
    outraw = nc.dram_tensor("outraw", [NCHUNK, U, BC], dt.float32, kind="ExternalOutput")
    lastout = nc.dram_tensor("lastout", [NCHUNK, U, BC], dt.float32, kind="ExternalOutput")

    with tile.TileContext(nc) as tc, ExitStack() as octx:
        singles = octx.enter_context(tc.tile_pool(name="singles", bufs=1))
        dram = octx.enter_context(tc.tile_pool(name="dram", bufs=1, space="DRAM"))

        def load_w(dram_w, p):
            t = singles.tile([p, U], mybir.dt.bfloat16, tag=dram_w.name)
            nc.sync.dma_start(out=t, in_=dram_w[:, :])
            return t
        KzN, Kr, Kh = load_w(wKzN, 101), load_w(wKr, 101), load_w(wKh, 101)
        RzN, Rr, Rh = load_w(wRzN, 101), load_w(wRr, 101), load_w(wRh, 101)
        A1b, A2b, Vr, I100 = load_w(wA1, U), load_w(wA2, U), load_w(wVr, U), load_w(wI, U)
        b0h = singles.tile([U, 1], mybir.dt.float32, tag="b0h")
        nc.sync.dma_start(out=b0h, in_=wb0h[:, :])

        state = dram.tile([NCHUNK, NB, U, G * BC], mybir.dt.bfloat16)

        # h blocks: 2 rotating buffers per chunk; row 100 is a constant 1.0
        # (carries recurrent biases through the matmuls). memset once.
        hblk = [[singles.tile([101, G, BC], mybir.dt.bfloat16, tag=f"hblk{c}_{i}",
                              name=f"hblk{c}_{i}")
                 for i in range(2)] for c in range(NCHUNK)]
        for c in range(NCHUNK):
            for i in range(2):
                nc.sync.dma_start(out=hblk[c][i][100:101, :, :], in_=wones[:, :])
        h0 = [singles.tile([101, BC], mybir.dt.bfloat16, tag=f"h0_{c}",
                           name=f"h0_{c}")
              for c in range(NCHUNK)]
        for c in range(NCHUNK):
            nc.vector.memset(h0[c], 0.0)
            nc.sync.dma_start(out=h0[c][100:101, :], in_=wones[:, 0:BC])

        last_tiles = []

        # ---------------- scan ----------------
        with ExitStack() as ctx:
            xp = ctx.enter_context(tc.tile_pool(name="xp", bufs=3))
            gp = ctx.enter_context(tc.tile_pool(name="gp", bufs=3))
            pzr = ctx.enter_context(tc.tile_pool(name="pzr", bufs=2, space="PSUM"))
            phx = ctx.enter_context(tc.tile_pool(name="phx", bufs=2, space="PSUM"))
            prh = ctx.enter_context(tc.tile_pool(name="prh", bufs=2, space="PSUM"))

            hprev = [h0[0], h0[1]]
            zr_t = {}
            hx_t = {}

            def proj(t):
                nb, g = divmod(t, G)
                xt = _xblks[nb][:, g, :]  # [101, 512]
                zr = pzr.tile([128, 2, 512], mybir.dt.float32, tag="zr")
                hx = phx.tile([128, 512], mybir.dt.float32, tag="hx")
                nc.tensor.matmul(zr[0:U, 0, :], lhsT=KzN, rhs=xt, start=True, stop=False)
                nc.tensor.matmul(zr[0:U, 1, :], lhsT=Kr, rhs=xt, start=True, stop=False)
                nc.tensor.matmul(hx[0:U, :], lhsT=Kh, rhs=xt, start=True, stop=True)
                zr_t[t] = zr
                hx_t[t] = hx

            _xblks = {}
            def load_xblk(nb):
                xb = xp.tile([101, G, 512], mybir.dt.bfloat16, tag="xblk")
                nc.sync.dma_start(out=xb, in_=xaug[nb, :, :])
                _xblks[nb] = xb

            load_xblk(0)
            proj(0)
            for t in range(T):
                nb, g = divmod(t, G)
                if g == 0 and nb + 1 < NB:
                    load_xblk(nb + 1)
                if t + 1 < T:
                    proj(t + 1)
                zr, hx = zr_t.pop(t), hx_t.pop(t)
                rh = prh.tile([128, 512], mybir.dt.float32, tag="rh")
                for c in range(NCHUNK):
                    h = hprev[c]
                    sl = slice(c * BC, (c + 1) * BC)
                    nc.tensor.matmul(zr[0:U, 0, sl], lhsT=RzN, rhs=h, start=False, stop=True)
                    nc.tensor.matmul(zr[0:U, 1, sl], lhsT=Rr, rhs=h, start=False, stop=True)
                    nc.tensor.matmul(rh[0:U, sl], lhsT=Rh, rhs=h, start=True, stop=True)
                for c in range(NCHUNK):
                    h = hprev[c]
                    sl = slice(c * BC, (c + 1) * BC)
                    # critical path: sigmoid(r) -> t1 -> s -> tanh -> q -> hn
                    rs = gp.tile([U, BC], mybir.dt.bfloat16, tag=f"rs{c}")
                    nc.scalar.activation(rs, zr[0:U, 1, sl], AF.Sigmoid)
                    zs = gp.tile([U, BC], mybir.dt.bfloat16, tag=f"zs{c}")
                    nc.scalar.activation(zs, zr[0:U, 0, sl], AF.Sigmoid)
                    t1 = gp.tile([U, BC], mybir.dt.bfloat16, tag=f"t1{c}")
                    nc.vector.tensor_tensor(t1, rs, rh[0:U, sl], OP.mult)
                    s = gp.tile([U, BC], mybir.dt.bfloat16, tag=f"s{c}")
                    nc.vector.tensor_tensor(s, t1, hx[0:U, sl], OP.add)
                    # p = (zs - 1) * h is off the critical path (GpSimd)
                    p = gp.tile([U, BC], mybir.dt.bfloat16, tag=f"p{c}")
                    nc.vector.scalar_tensor_tensor(p, zs, 1.0, h[0:U, :],
                                                   OP.subtract, OP.mult)
                    hh = gp.tile([U, BC], mybir.dt.bfloat16, tag=f"hh{c}")
                    nc.scalar.activation(hh, s, AF.Tanh, bias=b0h)
                    q = gp.tile([U, BC], mybir.dt.bfloat16, tag=f"q{c}")
                    nc.vector.tensor_tensor(q, zs, hh, OP.mult)
                    hb = hblk[c][nb % 2]
                    # hn = q - p = zs*hh + (1-zs)*h
                    nc.vector.tensor_tensor(hb[0:U, g, :], q, p, OP.subtract)
                    hprev[c] = hb[:, g, :]
                if g == G - 1:
                    for c in range(NCHUNK):
                        nc.sync.dma_start(out=state[c, nb, :, :],
                                          in_=hblk[c][nb % 2][0:U, :, :])

            for c in range(NCHUNK):
                lt = singles.tile([U, BC], mybir.dt.bfloat16, tag=f"last{c}")
                nc.vector.tensor_copy(lt, hprev[c][0:U, :])
                last_tiles.append(lt)
                lo = singles.tile([U, BC], mybir.dt.float32, tag=f"lasto{c}")
                nc.vector.tensor_copy(lo, hprev[c][0:U, :])
                nc.sync.dma_start(out=lastout[c, :, :], in_=lo)

        # ---------------- attention ----------------
        with ExitStack() as ctx:
            sp = ctx.enter_context(tc.tile_pool(name="sp", bufs=3))
            gp2 = ctx.enter_context(tc.tile_pool(name="gp2", bufs=3))
            ps = ctx.enter_context(tc.tile_pool(name="ps", bufs=2, space="PSUM"))
            pa = ctx.enter_context(tc.tile_pool(name="pa", bufs=2, space="PSUM"))
            po = ctx.enter_context(tc.tile_pool(name="po", bufs=1, space="PSUM"))

            for c in range(NCHUNK):
                # c1 = A1 @ last, replicated into both pair halves (bf16 SBUF)
                c1p = po.tile([128, 512], mybir.dt.float32, tag="c1p")
                nc.tensor.matmul(c1p[0:U, 0:BC], lhsT=A1b, rhs=last_tiles[c],
                                 start=True, stop=True)
                c1c = singles.tile([U, 2, BC], mybir.dt.bfloat16, tag=f"c1c{c}")
                nc.scalar.activation(c1c[:, 0, :], c1p[0:U, 0:BC], AF.Copy)
                nc.scalar.activation(c1c[:, 1, :], c1p[0:U, 0:BC], AF.Copy)
                acc = po.tile([128, 512], mybir.dt.float32, tag="acc")
                npair = T // 2
                for nb in range(NB):
                    stb = sp.tile([U, G, BC], mybir.dt.bfloat16, tag="stb")
                    nc.sync.dma_start(out=stb, in_=state[c, nb, :, :])
                    for p in range(G // 2):
                        pi = nb * (G // 2) + p
                        stpair = stb[:, 2 * p:2 * p + 2, :]
                        sb = ps.tile([128, 512], mybir.dt.float32, tag="sb")
                        nc.tensor.matmul(sb[0:U, :], lhsT=A2b, rhs=stpair, start=True, stop=True)
                        gin = gp2.tile([U, 2, BC], mybir.dt.bfloat16, tag="gin")
                        nc.vector.tensor_tensor(gin, c1c, sb[0:U, :], OP.add)
                        gt = gp2.tile([U, 2, BC], mybir.dt.bfloat16, tag="gt")
                        nc.scalar.activation(gt, gin, AF.Sigmoid)
                        al = pa.tile([128, 512], mybir.dt.float32, tag="al")
                        nc.tensor.matmul(al[0:U, :], lhsT=Vr, rhs=gt, start=True, stop=True)
                        tmp = gp2.tile([U, 2, BC], mybir.dt.bfloat16, tag="tmp")
                        nc.vector.tensor_tensor(tmp, al[0:U, :], stpair, OP.mult)
                        nc.tensor.matmul(acc[0:U, :], lhsT=I100, rhs=tmp,
                                         start=(pi == 0), stop=(pi == npair - 1))
                foldl = gp2.tile([U, BC], mybir.dt.float32, tag="foldl")
                nc.scalar.activation(foldl, acc[0:U, 0:BC], AF.Copy)
                osb = gp2.tile([U, BC], mybir.dt.float32, tag="osb")
                nc.vector.tensor_tensor(osb, foldl, acc[0:U, BC:2 * BC], OP.add)
                nc.sync.dma_start(out=outraw[c, :, :], in_=osb)

    nc.compile()
    return nc


def _prep_weights(kernel_w, rec_kernel, bias, A1_w, A2_w, v):
    b0, b1 = bias[0], bias[1]
    w = {}
    KzN = np.zeros((101, U), np.float32)
    KzN[:E] = -kernel_w[:, :U]
    KzN[100, :] = -40.0
    Kr = np.zeros((101, U), np.float32)
    Kr[:E] = kernel_w[:, U:2 * U]
    Kh = np.zeros((101, U), np.float32)
    Kh[:E] = kernel_w[:, 2 * U:]
    RzN = np.zeros((101, U), np.float32)
    RzN[:U] = -rec_kernel[:, :U]
    RzN[100, :] = -(b0[:U] + b1[:U])
    Rr = np.zeros((101, U), np.float32)
    Rr[:U] = rec_kernel[:, U:2 * U]
    Rr[100, :] = b0[U:2 * U] + b1[U:2 * U]
    Rh = np.zeros((101, U), np.float32)
    Rh[:U] = rec_kernel[:, 2 * U:]
    Rh[100, :] = b1[2 * U:]
    w["wKzN"], w["wKr"], w["wKh"] = KzN, Kr, Kh
    w["wRzN"], w["wRr"], w["wRh"] = RzN, Rr, Rh
    w = {k: val.astype(bf16) for k, val in w.items()}
    w["wb0h"] = b0[2 * U:][:, None].astype(np.float32)
    w["wones"] = np.ones((1, G * BC), bf16)
    w["wA1"] = A1_w.astype(bf16)
    w["wA2"] = A2_w.astype(bf16)
    w["wVr"] = np.broadcast_to(v[0][:, None], (U, U)).astype(bf16).copy()
    w["wI"] = np.eye(U, dtype=np.float32).astype(bf16)
    return w


def kernel(session_hidden, mask, kernel, rec_kernel, bias, A1_w, A2_w, v, _trace=False):
    session_hidden = np.asarray(session_hidden, np.float32)
    mask = np.asarray(mask, np.float32)
    kernel_w = np.asarray(kernel, np.float32)
    rec_kernel = np.asarray(rec_kernel, np.float32)
    bias = np.asarray(bias, np.float32)
    A1_w = np.asarray(A1_w, np.float32)
    A2_w = np.asarray(A2_w, np.float32)
    v = np.asarray(v, np.float32)

    if "nc" not in _CACHE:
        _CACHE["nc"] = _build()
    nc = _CACHE["nc"]

    w = _prep_weights(kernel_w, rec_kernel, bias, A1_w, A2_w, v)

    # xaug: [NB, 101, G, NCHUNK, BC] flattened; rows 0:100 = x^T, row 100 = 1-m
    x = session_hidden.reshape(NCORES, NCHUNK, BC, T, E)
    m = mask.reshape(NCORES, NCHUNK, BC, T)
    in_maps = []
    for k in range(NCORES):
        xa = np.zeros((NB, 101, G, NCHUNK, BC), np.float32)
        # x[k]: [NCHUNK, BC, T, E] -> [T, E, NCHUNK, BC] -> [NB, G, E, NCHUNK, BC]
        xt = x[k].transpose(2, 3, 0, 1).reshape(NB, G, E, NCHUNK, BC)
        xa[:, :E] = xt.transpose(0, 2, 1, 3, 4)
        mm_ = (1.0 - m[k]).transpose(2, 0, 1).reshape(NB, G, NCHUNK, BC)
        xa[:, 100] = mm_
        im = dict(w)
        im["xaug"] = xa.reshape(NB, 101, G * NCHUNK * BC).astype(bf16)
        in_maps.append(im)

    res = bass_utils.run_bass_kernel_spmd(nc, in_maps, core_ids=list(range(NCORES)),
                                          trace=_trace)
    _CACHE["last_res"] = res

    out_raw = np.zeros((B, U), np.float32)
    last = np.zeros((B, U), np.float32)
    for k in range(NCORES):
        r = res.results[k]
        for c in range(NCHUNK):
            sl = slice(k * PERCORE + c * BC, k * PERCORE + (c + 1) * BC)
            out_raw[sl] = np.asarray(r["outraw"][c]).T.astype(np.float32)
            last[sl] = np.asarray(r["lastout"][c]).T.astype(np.float32)

    # host correction for masked timesteps (device used last@A1 term for ALL t)
    lengths = mask.sum(1)
    sl_ = last @ A2_w
    c_ = last @ A1_w
    sig = lambda a: 1.0 / (1.0 + np.exp(-a))
    a_corr = (sig(sl_ + c_) - sig(sl_)) @ v[0]
    out = out_raw - (T - lengths)[:, None] * a_corr[:, None] * last
    return out.astype(np.float32)


# revision 16
# speedup vs baseline: 1.1768x; 1.1768x over previous
"""Trainium2 Bass kernel for nn_LocalEncoder (masked GRU + attention pooling).

v2 strategy (data-parallel over batch, 8 cores x 512 rows, 2 chunks of 256):
- Feature-major layout [U partitions, batch free]; bf16 matmuls -> fp32 PSUM.
- Scan: x-projections for BOTH chunks fused 512-wide (chunks side by side in
  the free dim); recurrent matmuls 256-wide per chunk accumulate into halves.
  No bias matmuls: z/r/h-recurrent biases ride a constant-ones row (row 100)
  of the h tile; the candidate x-bias rides scalar_tensor_tensor's
  per-partition scalar. Trailing-padding mask folded via xaug row 100
  (1-m, scaled -40 in the z weights) freezing h exactly like the reference.
- DMAs batched over G=8 timesteps (xaug in, state out, state in).
- d = hh - h runs on GpSimd to unload the Vector engine.
- Attention: 512-wide timestep pairs; A1@[last|last] accumulated in PSUM;
  alpha*state accumulated via identity-matmul into a 512-wide PSUM bank,
  folded once at the end. Device uses last@A1 for ALL t; host subtracts the
  closed-form correction for masked (trailing) timesteps.
"""
import sys
sys.path.insert(0, "/opt/trn_rl_repo")
from contextlib import ExitStack

import numpy as np
import ml_dtypes

import concourse.bass as bass
import concourse.bacc as bacc
import concourse.tile as tile
from concourse import mybir
from concourse import bass_utils

bf16 = ml_dtypes.bfloat16
AF = mybir.ActivationFunctionType
OP = mybir.AluOpType

B, T, E, U = 4096, 200, 100, 100
NCORES = 8
BC = 256          # chunk width
NCHUNK = 2
PERCORE = BC * NCHUNK
G = 8             # timesteps per DMA block
NB = T // G

_CACHE = {}


def _build():
    nc = bacc.Bacc()
    dt = mybir.dt
    xaug = nc.dram_tensor("xaug", [NB, 101, G * 512], dt.bfloat16, kind="ExternalInput")
    wKzN = nc.dram_tensor("wKzN", [101, U], dt.bfloat16, kind="ExternalInput")
    wKr = nc.dram_tensor("wKr", [101, U], dt.bfloat16, kind="ExternalInput")
    wKh = nc.dram_tensor("wKh", [101, U], dt.bfloat16, kind="ExternalInput")
    wRzN = nc.dram_tensor("wRzN", [101, U], dt.bfloat16, kind="ExternalInput")
    wRr = nc.dram_tensor("wRr", [101, U], dt.bfloat16, kind="ExternalInput")
    wRh = nc.dram_tensor("wRh", [101, U], dt.bfloat16, kind="ExternalInput")
    wb0h = nc.dram_tensor("wb0h", [U, 1], dt.float32, kind="ExternalInput")
    wones = nc.dram_tensor("wones", [1, G * BC], dt.bfloat16, kind="ExternalInput")
    wA1 = nc.dram_tensor("wA1", [U, U], dt.bfloat16, kind="ExternalInput")
    wA2 = nc.dram_tensor("wA2", [U, U], dt.bfloat16, kind="ExternalInput")
    wVr = nc.dram_tensor("wVr", [U, U], dt.bfloat16, kind="ExternalInput")
    wI = nc.dram_tensor("wI", [U, U], dt.bfloat16, kind="ExternalInput")
    outraw = nc.dram_tensor("outraw", [NCHUNK, U, BC], dt.float32, kind="ExternalOutput")
    lastout = nc.dram_tensor("lastout", [NCHUNK, U, BC], dt.float32, kind="ExternalOutput")

    with tile.TileContext(nc) as tc, ExitStack() as octx:
        singles = octx.enter_context(tc.tile_pool(name="singles", bufs=1))
        dram = octx.enter_context(tc.tile_pool(name="dram", bufs=1, space="DRAM"))

        def load_w(dram_w, p):
            t = singles.tile([p, U], mybir.dt.bfloat16, tag=dram_w.name)
            nc.sync.dma_start(out=t, in_=dram_w[:, :])
            return t
        KzN, Kr, Kh = load_w(wKzN, 101), load_w(wKr, 101), load_w(wKh, 101)
        RzN, Rr, Rh = load_w(wRzN, 101), load_w(wRr, 101), load_w(wRh, 101)
        A1b, A2b, Vr, I100 = load_w(wA1, U), load_w(wA2, U), load_w(wVr, U), load_w(wI, U)
        b0h = singles.tile([U, 1], mybir.dt.float32, tag="b0h")
        nc.sync.dma_start(out=b0h, in_=wb0h[:, :])

        state = dram.tile([NCHUNK, NB, U, G * BC], mybir.dt.bfloat16)

        # h blocks: 2 rotating buffers per chunk; row 100 is a constant 1.0
        # (carries recurrent biases through the matmuls). memset once.
        hblk = [[singles.tile([101, G, BC], mybir.dt.bfloat16, tag=f"hblk{c}_{i}",
                              name=f"hblk{c}_{i}")
                 for i in range(2)] for c in range(NCHUNK)]
        for c in range(NCHUNK):
            for i in range(2):
                nc.sync.dma_start(out=hblk[c][i][100:101, :, :], in_=wones[:, :])
        h0 = [singles.tile([101, BC], mybir.dt.bfloat16, tag=f"h0_{c}",
                           name=f"h0_{c}")
              for c in range(NCHUNK)]
        for c in range(NCHUNK):
            nc.vector.memset(h0[c], 0.0)
            nc.sync.dma_start(out=h0[c][100:101, :], in_=wones[:, 0:BC])

        last_tiles = []

        # ---------------- scan ----------------
        with ExitStack() as ctx:
            xp = ctx.enter_context(tc.tile_pool(name="xp", bufs=3))
            gp = ctx.enter_context(tc.tile_pool(name="gp", bufs=3))
            # per-chunk single-buffered PSUM tiles: r/z/rh split so the first
            # sigmoid only waits on its own two matmuls (tile-level deps)
            prr = ctx.enter_context(tc.tile_pool(name="prr", bufs=1, space="PSUM"))
            pzz = ctx.enter_context(tc.tile_pool(name="pzz", bufs=1, space="PSUM"))
            prh = ctx.enter_context(tc.tile_pool(name="prh", bufs=1, space="PSUM"))
            phx = ctx.enter_context(tc.tile_pool(name="phx", bufs=2, space="PSUM"))

            hprev = [h0[0], h0[1]]
            rr_t = {}
            zz_t = {}
            hx_t = {}

            def proj(t):
                # x-projections for step t (emitted after step t-1's recurrent
                # burst; single-buffered PSUM WARs resolve mid-period)
                nb, g = divmod(t, G)
                hx = phx.tile([128, 512], mybir.dt.float32, tag="hx")
                for c in range(NCHUNK):
                    xc = _xblks[nb][:, g, c * BC:(c + 1) * BC]  # [101, 256]
                    rr = prr.tile([128, BC], mybir.dt.float32, tag=f"rr{c}")
                    zz = pzz.tile([128, BC], mybir.dt.float32, tag=f"zz{c}")
                    nc.tensor.matmul(rr[0:U, :], lhsT=Kr, rhs=xc, start=True, stop=False)
                    nc.tensor.matmul(zz[0:U, :], lhsT=KzN, rhs=xc, start=True, stop=False)
                    rr_t[(t, c)] = rr
                    zz_t[(t, c)] = zz
                xt = _xblks[nb][:, g, :]  # [101, 512]
                nc.tensor.matmul(hx[0:U, :], lhsT=Kh, rhs=xt, start=True, stop=True)
                hx_t[t] = hx

            _xblks = {}
            def load_xblk(nb):
                xb = xp.tile([101, G, 512], mybir.dt.bfloat16, tag="xblk")
                nc.sync.dma_start(out=xb, in_=xaug[nb, :, :])
                _xblks[nb] = xb

            load_xblk(0)
            proj(0)
            for t in range(T):
                nb, g = divmod(t, G)
                if g == 0 and nb + 1 < NB:
                    load_xblk(nb + 1)
                hx = hx_t.pop(t)
                for c in range(NCHUNK):
                    h = hprev[c]
                    sl = slice(c * BC, (c + 1) * BC)
                    rr, zz = rr_t.pop((t, c)), zz_t.pop((t, c))
                    rh = prh.tile([128, BC], mybir.dt.float32, tag=f"rh{c}")
                    # critical path: Rr@h -> sigmoid(r) -> t1 -> s -> tanh -> q -> hn
                    nc.tensor.matmul(rr[0:U, :], lhsT=Rr, rhs=h, start=False, stop=True)
                    nc.tensor.matmul(zz[0:U, :], lhsT=RzN, rhs=h, start=False, stop=True)
                    nc.tensor.matmul(rh[0:U, :], lhsT=Rh, rhs=h, start=True, stop=True)
                    rs = gp.tile([U, BC], mybir.dt.bfloat16, tag=f"rs{c}")
                    nc.scalar.activation(rs, rr[0:U, :], AF.Sigmoid)
                    zs = gp.tile([U, BC], mybir.dt.bfloat16, tag=f"zs{c}")
                    nc.scalar.activation(zs, zz[0:U, :], AF.Sigmoid)
                    t1 = gp.tile([U, BC], mybir.dt.bfloat16, tag=f"t1{c}")
                    nc.vector.tensor_tensor(t1, rs, rh[0:U, :], OP.mult)
                    s = gp.tile([U, BC], mybir.dt.bfloat16, tag=f"s{c}")
                    nc.vector.tensor_tensor(s, t1, hx[0:U, sl], OP.add)
                    # p = (zs - 1) * h is off the critical path
                    p = gp.tile([U, BC], mybir.dt.bfloat16, tag=f"p{c}")
                    nc.vector.scalar_tensor_tensor(p, zs, 1.0, h[0:U, :],
                                                   OP.subtract, OP.mult)
                    hh = gp.tile([U, BC], mybir.dt.bfloat16, tag=f"hh{c}")
                    nc.scalar.activation(hh, s, AF.Tanh, bias=b0h)
                    q = gp.tile([U, BC], mybir.dt.bfloat16, tag=f"q{c}")
                    nc.vector.tensor_tensor(q, zs, hh, OP.mult)
                    hb = hblk[c][nb % 2]
                    # hn = q - p = zs*hh + (1-zs)*h
                    nc.vector.tensor_tensor(hb[0:U, g, :], q, p, OP.subtract)
                    hprev[c] = hb[:, g, :]
                if t + 1 < T:
                    proj(t + 1)
                if g == G - 1:
                    for c in range(NCHUNK):
                        nc.sync.dma_start(out=state[c, nb, :, :],
                                          in_=hblk[c][nb % 2][0:U, :, :])

            for c in range(NCHUNK):
                lt = singles.tile([U, BC], mybir.dt.bfloat16, tag=f"last{c}")
                nc.vector.tensor_copy(lt, hprev[c][0:U, :])
                last_tiles.append(lt)
                lo = singles.tile([U, BC], mybir.dt.float32, tag=f"lasto{c}")
                nc.vector.tensor_copy(lo, hprev[c][0:U, :])
                nc.sync.dma_start(out=lastout[c, :, :], in_=lo)

        # ---------------- attention ----------------
        with ExitStack() as ctx:
            sp = ctx.enter_context(tc.tile_pool(name="sp", bufs=3))
            gp2 = ctx.enter_context(tc.tile_pool(name="gp2", bufs=3))
            ps = ctx.enter_context(tc.tile_pool(name="ps", bufs=2, space="PSUM"))
            pa = ctx.enter_context(tc.tile_pool(name="pa", bufs=2, space="PSUM"))
            po = ctx.enter_context(tc.tile_pool(name="po", bufs=1, space="PSUM"))

            for c in range(NCHUNK):
                lp = singles.tile([U, 2, BC], mybir.dt.bfloat16, tag=f"lp{c}")
                nc.vector.tensor_copy(lp[:, 0, :], last_tiles[c])
                nc.vector.tensor_copy(lp[:, 1, :], last_tiles[c])
                acc = po.tile([128, 512], mybir.dt.float32, tag="acc")
                npair = T // 2
                for nb in range(NB):
                    stb = sp.tile([U, G, BC], mybir.dt.bfloat16, tag="stb")
                    nc.gpsimd.dma_start(out=stb, in_=state[c, nb, :, :])
                    for p in range(G // 2):
                        pi = nb * (G // 2) + p
                        stpair = stb[:, 2 * p:2 * p + 2, :]
                        sb = ps.tile([128, 512], mybir.dt.float32, tag="sb")
                        nc.tensor.matmul(sb[0:U, :], lhsT=A2b, rhs=stpair, start=True, stop=False)
                        nc.tensor.matmul(sb[0:U, :], lhsT=A1b, rhs=lp, start=False, stop=True)
                        gt = gp2.tile([U, 2, BC], mybir.dt.bfloat16, tag="gt")
                        nc.scalar.activation(gt, sb[0:U, :], AF.Sigmoid)
                        al = pa.tile([128, 512], mybir.dt.float32, tag="al")
                        nc.tensor.matmul(al[0:U, :], lhsT=Vr, rhs=gt, start=True, stop=True)
                        tmp = gp2.tile([U, 2, BC], mybir.dt.bfloat16, tag="tmp")
                        nc.vector.tensor_tensor(tmp, al[0:U, :], stpair, OP.mult)
                        nc.tensor.matmul(acc[0:U, :], lhsT=I100, rhs=tmp,
                                         start=(pi == 0), stop=(pi == npair - 1))
                foldl = gp2.tile([U, BC], mybir.dt.float32, tag="foldl")
                nc.scalar.activation(foldl, acc[0:U, 0:BC], AF.Copy)
                osb = gp2.tile([U, BC], mybir.dt.float32, tag="osb")
                nc.vector.tensor_tensor(osb, foldl, acc[0:U, BC:2 * BC], OP.add)
                nc.sync.dma_start(out=outraw[c, :, :], in_=osb)

    nc.compile()
    return nc


def _prep_weights(kernel_w, rec_kernel, bias, A1_w, A2_w, v):
    b0, b1 = bias[0], bias[1]
    w = {}
    KzN = np.zeros((101, U), np.float32)
    KzN[:E] = -kernel_w[:, :U]
    KzN[100, :] = -40.0
    Kr = np.zeros((101, U), np.float32)
    Kr[:E] = kernel_w[:, U:2 * U]
    Kh = np.zeros((101, U), np.float32)
    Kh[:E] = kernel_w[:, 2 * U:]
    RzN = np.zeros((101, U), np.float32)
    RzN[:U] = -rec_kernel[:, :U]
    RzN[100, :] = -(b0[:U] + b1[:U])
    Rr = np.zeros((101, U), np.float32)
    Rr[:U] = rec_kernel[:, U:2 * U]
    Rr[100, :] = b0[U:2 * U] + b1[U:2 * U]
    Rh = np.zeros((101, U), np.float32)
    Rh[:U] = rec_kernel[:, 2 * U:]
    Rh[100, :] = b1[2 * U:]
    w["wKzN"], w["wKr"], w["wKh"] = KzN, Kr, Kh
    w["wRzN"], w["wRr"], w["wRh"] = RzN, Rr, Rh
    w = {k: val.astype(bf16) for k, val in w.items()}
    w["wb0h"] = b0[2 * U:][:, None].astype(np.float32)
    w["wones"] = np.ones((1, G * BC), bf16)
    w["wA1"] = A1_w.astype(bf16)
    w["wA2"] = A2_w.astype(bf16)
    w["wVr"] = np.broadcast_to(v[0][:, None], (U, U)).astype(bf16).copy()
    w["wI"] = np.eye(U, dtype=np.float32).astype(bf16)
    return w


def kernel(session_hidden, mask, kernel, rec_kernel, bias, A1_w, A2_w, v, _trace=False):
    session_hidden = np.asarray(session_hidden, np.float32)
    mask = np.asarray(mask, np.float32)
    kernel_w = np.asarray(kernel, np.float32)
    rec_kernel = np.asarray(rec_kernel, np.float32)
    bias = np.asarray(bias, np.float32)
    A1_w = np.asarray(A1_w, np.float32)
    A2_w = np.asarray(A2_w, np.float32)
    v = np.asarray(v, np.float32)

    if "nc" not in _CACHE:
        _CACHE["nc"] = _build()
    nc = _CACHE["nc"]

    w = _prep_weights(kernel_w, rec_kernel, bias, A1_w, A2_w, v)

    # xaug: [NB, 101, G, NCHUNK, BC] flattened; rows 0:100 = x^T, row 100 = 1-m
    x = session_hidden.reshape(NCORES, NCHUNK, BC, T, E)
    m = mask.reshape(NCORES, NCHUNK, BC, T)
    in_maps = []
    for k in range(NCORES):
        xa = np.zeros((NB, 101, G, NCHUNK, BC), np.float32)
        # x[k]: [NCHUNK, BC, T, E] -> [T, E, NCHUNK, BC] -> [NB, G, E, NCHUNK, BC]
        xt = x[k].transpose(2, 3, 0, 1).reshape(NB, G, E, NCHUNK, BC)
        xa[:, :E] = xt.transpose(0, 2, 1, 3, 4)
        mm_ = (1.0 - m[k]).transpose(2, 0, 1).reshape(NB, G, NCHUNK, BC)
        xa[:, 100] = mm_
        im = dict(w)
        im["xaug"] = xa.reshape(NB, 101, G * NCHUNK * BC).astype(bf16)
        in_maps.append(im)

    res = bass_utils.run_bass_kernel_spmd(nc, in_maps, core_ids=list(range(NCORES)),
                                          trace=_trace)
    _CACHE["last_res"] = res

    out_raw = np.zeros((B, U), np.float32)
    last = np.zeros((B, U), np.float32)
    for k in range(NCORES):
        r = res.results[k]
        for c in range(NCHUNK):
            sl = slice(k * PERCORE + c * BC, k * PERCORE + (c + 1) * BC)
            out_raw[sl] = np.asarray(r["outraw"][c]).T.astype(np.float32)
            last[sl] = np.asarray(r["lastout"][c]).T.astype(np.float32)

    # host correction for masked timesteps (device used last@A1 term for ALL t)
    lengths = mask.sum(1)
    sl_ = last @ A2_w
    c_ = last @ A1_w
    sig = lambda a: 1.0 / (1.0 + np.exp(-a))
    a_corr = (sig(sl_ + c_) - sig(sl_)) @ v[0]
    out = out_raw - (T - lengths)[:, None] * a_corr[:, None] * last
    return out.astype(np.float32)


# revision 17
# speedup vs baseline: 1.2386x; 1.0525x over previous
"""Trainium2 Bass kernel for nn_LocalEncoder (masked GRU + attention pooling).

v2 strategy (data-parallel over batch, 8 cores x 512 rows, 2 chunks of 256):
- Feature-major layout [U partitions, batch free]; bf16 matmuls -> fp32 PSUM.
- Scan: x-projections for BOTH chunks fused 512-wide (chunks side by side in
  the free dim); recurrent matmuls 256-wide per chunk accumulate into halves.
  No bias matmuls: z/r/h-recurrent biases ride a constant-ones row (row 100)
  of the h tile; the candidate x-bias rides scalar_tensor_tensor's
  per-partition scalar. Trailing-padding mask folded via xaug row 100
  (1-m, scaled -40 in the z weights) freezing h exactly like the reference.
- DMAs batched over G=8 timesteps (xaug in, state out, state in).
- d = hh - h runs on GpSimd to unload the Vector engine.
- Attention: 512-wide timestep pairs; A1@[last|last] accumulated in PSUM;
  alpha*state accumulated via identity-matmul into a 512-wide PSUM bank,
  folded once at the end. Device uses last@A1 for ALL t; host subtracts the
  closed-form correction for masked (trailing) timesteps.
"""
import sys
sys.path.insert(0, "/opt/trn_rl_repo")
from contextlib import ExitStack

import numpy as np
import ml_dtypes

import concourse.bass as bass
import concourse.bacc as bacc
import concourse.tile as tile
from concourse import mybir
from concourse import bass_utils

bf16 = ml_dtypes.bfloat16
AF = mybir.ActivationFunctionType
OP = mybir.AluOpType

B, T, E, U = 4096, 200, 100, 100
NCORES = 8
BC = 256          # chunk width
NCHUNK = 2
PERCORE = BC * NCHUNK
G = 8             # timesteps per DMA block
NB = T // G

_CACHE = {}


def _build():
    nc = bacc.Bacc()
    dt = mybir.dt
    xaug = nc.dram_tensor("xaug", [NB, 101, G * 512], dt.bfloat16, kind="ExternalInput")
    wKzN = nc.dram_tensor("wKzN", [101, U], dt.bfloat16, kind="ExternalInput")
    wKr = nc.dram_tensor("wKr", [101, U], dt.bfloat16, kind="ExternalInput")
    wKh = nc.dram_tensor("wKh", [101, U], dt.bfloat16, kind="ExternalInput")
    wRzN = nc.dram_tensor("wRzN", [101, U], dt.bfloat16, kind="ExternalInput")
    wRr = nc.dram_tensor("wRr", [101, U], dt.bfloat16, kind="ExternalInput")
    wRh = nc.dram_tensor("wRh", [101, U], dt.bfloat16, kind="ExternalInput")
    wb0h = nc.dram_tensor("wb0h", [U, 1], dt.float32, kind="ExternalInput")
    wones = nc.dram_tensor("wones", [1, G * BC], dt.bfloat16, kind="ExternalInput")
    wA1 = nc.dram_tensor("wA1", [U, U], dt.bfloat16, kind="ExternalInput")
    wA2 = nc.dram_tensor("wA2", [U, U], dt.bfloat16, kind="ExternalInput")
    wVr = nc.dram_tensor("wVr", [U, U], dt.bfloat16, kind="ExternalInput")
    wI = nc.dram_tensor("wI", [U, U], dt.bfloat16, kind="ExternalInput")
    outraw = nc.dram_tensor("outraw", [NCHUNK, U, BC], dt.float32, kind="ExternalOutput")
    lastout = nc.dram_tensor("lastout", [NCHUNK, U, BC], dt.float32, kind="ExternalOutput")

    with tile.TileContext(nc) as tc, ExitStack() as octx:
        singles = octx.enter_context(tc.tile_pool(name="singles", bufs=1))
        dram = octx.enter_context(tc.tile_pool(name="dram", bufs=1, space="DRAM"))

        def load_w(dram_w, p):
            t = singles.tile([p, U], mybir.dt.bfloat16, tag=dram_w.name)
            nc.sync.dma_start(out=t, in_=dram_w[:, :])
            return t
        KzN, Kr, Kh = load_w(wKzN, 101), load_w(wKr, 101), load_w(wKh, 101)
        RzN, Rr, Rh = load_w(wRzN, 101), load_w(wRr, 101), load_w(wRh, 101)
        A1b, A2b, Vr, I100 = load_w(wA1, U), load_w(wA2, U), load_w(wVr, U), load_w(wI, U)
        b0h = singles.tile([U, 1], mybir.dt.float32, tag="b0h")
        nc.sync.dma_start(out=b0h, in_=wb0h[:, :])

        state = dram.tile([NCHUNK, NB, U, G * BC], mybir.dt.bfloat16)

        # h blocks: 2 rotating buffers per chunk; row 100 is a constant 1.0
        # (carries recurrent biases through the matmuls). memset once.
        hblk = [[singles.tile([101, G, BC], mybir.dt.bfloat16, tag=f"hblk{c}_{i}",
                              name=f"hblk{c}_{i}")
                 for i in range(2)] for c in range(NCHUNK)]
        for c in range(NCHUNK):
            for i in range(2):
                nc.sync.dma_start(out=hblk[c][i][100:101, :, :], in_=wones[:, :])
        h0 = [singles.tile([101, BC], mybir.dt.bfloat16, tag=f"h0_{c}",
                           name=f"h0_{c}")
              for c in range(NCHUNK)]
        for c in range(NCHUNK):
            nc.vector.memset(h0[c], 0.0)
            nc.sync.dma_start(out=h0[c][100:101, :], in_=wones[:, 0:BC])

        last_tiles = []

        # ---------------- scan ----------------
        with ExitStack() as ctx:
            xp = ctx.enter_context(tc.tile_pool(name="xp", bufs=3))
            gp = ctx.enter_context(tc.tile_pool(name="gp", bufs=3))
            # per-chunk single-buffered PSUM tiles: r/z/rh split so the first
            # sigmoid only waits on its own two matmuls (tile-level deps)
            prr = ctx.enter_context(tc.tile_pool(name="prr", bufs=1, space="PSUM"))
            pzz = ctx.enter_context(tc.tile_pool(name="pzz", bufs=1, space="PSUM"))
            prh = ctx.enter_context(tc.tile_pool(name="prh", bufs=1, space="PSUM"))
            phx = ctx.enter_context(tc.tile_pool(name="phx", bufs=1, space="PSUM"))

            hprev = [h0[0], h0[1]]
            rr_t = {}
            zz_t = {}
            hx_t = {}

            def proj(t):
                # x-projections for step t (emitted after step t-1's recurrent
                # burst; single-buffered per-chunk PSUM WARs resolve mid-period)
                nb, g = divmod(t, G)
                for c in range(NCHUNK):
                    xc = _xblks[nb][:, g, c * BC:(c + 1) * BC]  # [101, 256]
                    rr = prr.tile([128, BC], mybir.dt.float32, tag=f"rr{c}")
                    zz = pzz.tile([128, BC], mybir.dt.float32, tag=f"zz{c}")
                    hx = phx.tile([128, BC], mybir.dt.float32, tag=f"hx{c}")
                    nc.tensor.matmul(rr[0:U, :], lhsT=Kr, rhs=xc, start=True, stop=False)
                    nc.tensor.matmul(zz[0:U, :], lhsT=KzN, rhs=xc, start=True, stop=False)
                    nc.tensor.matmul(hx[0:U, :], lhsT=Kh, rhs=xc, start=True, stop=True)
                    rr_t[(t, c)] = rr
                    zz_t[(t, c)] = zz
                    hx_t[(t, c)] = hx

            _xblks = {}
            def load_xblk(nb):
                xb = xp.tile([101, G, 512], mybir.dt.bfloat16, tag="xblk")
                nc.sync.dma_start(out=xb, in_=xaug[nb, :, :])
                _xblks[nb] = xb

            load_xblk(0)
            proj(0)
            for t in range(T):
                nb, g = divmod(t, G)
                if g == 0 and nb + 1 < NB:
                    load_xblk(nb + 1)
                for c in range(NCHUNK):
                    h = hprev[c]
                    rr, zz = rr_t.pop((t, c)), zz_t.pop((t, c))
                    hx = hx_t.pop((t, c))
                    rh = prh.tile([128, BC], mybir.dt.float32, tag=f"rh{c}")
                    # critical path: Rr@h -> sigmoid(r) -> t1 -> s -> tanh -> q -> hn
                    nc.tensor.matmul(rr[0:U, :], lhsT=Rr, rhs=h, start=False, stop=True)
                    nc.tensor.matmul(zz[0:U, :], lhsT=RzN, rhs=h, start=False, stop=True)
                    nc.tensor.matmul(rh[0:U, :], lhsT=Rh, rhs=h, start=True, stop=True)
                    rs = gp.tile([U, BC], mybir.dt.bfloat16, tag=f"rs{c}")
                    nc.scalar.activation(rs, rr[0:U, :], AF.Sigmoid)
                    zs = gp.tile([U, BC], mybir.dt.bfloat16, tag=f"zs{c}")
                    nc.scalar.activation(zs, zz[0:U, :], AF.Sigmoid)
                    t1 = gp.tile([U, BC], mybir.dt.bfloat16, tag=f"t1{c}")
                    nc.vector.tensor_tensor(t1, rs, rh[0:U, :], OP.mult)
                    s = gp.tile([U, BC], mybir.dt.bfloat16, tag=f"s{c}")
                    nc.vector.tensor_tensor(s, t1, hx[0:U, :], OP.add)
                    # p = (zs - 1) * h is off the critical path
                    p = gp.tile([U, BC], mybir.dt.bfloat16, tag=f"p{c}")
                    nc.vector.scalar_tensor_tensor(p, zs, 1.0, h[0:U, :],
                                                   OP.subtract, OP.mult)
                    hh = gp.tile([U, BC], mybir.dt.bfloat16, tag=f"hh{c}")
                    nc.scalar.activation(hh, s, AF.Tanh, bias=b0h)
                    q = gp.tile([U, BC], mybir.dt.bfloat16, tag=f"q{c}")
                    nc.vector.tensor_tensor(q, zs, hh, OP.mult)
                    hb = hblk[c][nb % 2]
                    # hn = q - p = zs*hh + (1-zs)*h
                    nc.vector.tensor_tensor(hb[0:U, g, :], q, p, OP.subtract)
                    hprev[c] = hb[:, g, :]
                if t + 1 < T:
                    proj(t + 1)
                if g == G - 1:
                    for c in range(NCHUNK):
                        nc.gpsimd.dma_start(out=state[c, nb, :, :],
                                            in_=hblk[c][nb % 2][0:U, :, :])

            for c in range(NCHUNK):
                lt = singles.tile([U, BC], mybir.dt.bfloat16, tag=f"last{c}")
                nc.vector.tensor_copy(lt, hprev[c][0:U, :])
                last_tiles.append(lt)
                lo = singles.tile([U, BC], mybir.dt.float32, tag=f"lasto{c}")
                nc.vector.tensor_copy(lo, hprev[c][0:U, :])
                nc.sync.dma_start(out=lastout[c, :, :], in_=lo)

        # ---------------- attention ----------------
        with ExitStack() as ctx:
            sp = ctx.enter_context(tc.tile_pool(name="sp", bufs=3))
            gp2 = ctx.enter_context(tc.tile_pool(name="gp2", bufs=3))
            ps = ctx.enter_context(tc.tile_pool(name="ps", bufs=2, space="PSUM"))
            pa = ctx.enter_context(tc.tile_pool(name="pa", bufs=2, space="PSUM"))
            po = ctx.enter_context(tc.tile_pool(name="po", bufs=1, space="PSUM"))

            for c in range(NCHUNK):
                lp = singles.tile([U, 2, BC], mybir.dt.bfloat16, tag=f"lp{c}")
                nc.vector.tensor_copy(lp[:, 0, :], last_tiles[c])
                nc.vector.tensor_copy(lp[:, 1, :], last_tiles[c])
                acc = po.tile([128, 512], mybir.dt.float32, tag="acc")
                npair = T // 2
                for nb in range(NB):
                    stb = sp.tile([U, G, BC], mybir.dt.bfloat16, tag="stb")
                    eng = nc.sync if nb % 2 == 0 else nc.gpsimd
                    eng.dma_start(out=stb, in_=state[c, nb, :, :])
                    for p in range(G // 2):
                        pi = nb * (G // 2) + p
                        stpair = stb[:, 2 * p:2 * p + 2, :]
                        sb = ps.tile([128, 512], mybir.dt.float32, tag="sb")
                        nc.tensor.matmul(sb[0:U, :], lhsT=A2b, rhs=stpair, start=True, stop=False)
                        nc.tensor.matmul(sb[0:U, :], lhsT=A1b, rhs=lp, start=False, stop=True)
                        gt = gp2.tile([U, 2, BC], mybir.dt.bfloat16, tag="gt")
                        nc.scalar.activation(gt, sb[0:U, :], AF.Sigmoid)
                        al = pa.tile([128, 512], mybir.dt.float32, tag="al")
                        nc.tensor.matmul(al[0:U, :], lhsT=Vr, rhs=gt, start=True, stop=True)
                        tmp = gp2.tile([U, 2, BC], mybir.dt.bfloat16, tag="tmp")
                        nc.vector.tensor_tensor(tmp, al[0:U, :], stpair, OP.mult)
                        nc.tensor.matmul(acc[0:U, :], lhsT=I100, rhs=tmp,
                                         start=(pi == 0), stop=(pi == npair - 1))
                foldl = gp2.tile([U, BC], mybir.dt.float32, tag="foldl")
                nc.scalar.activation(foldl, acc[0:U, 0:BC], AF.Copy)
                osb = gp2.tile([U, BC], mybir.dt.float32, tag="osb")
                nc.vector.tensor_tensor(osb, foldl, acc[0:U, BC:2 * BC], OP.add)
                nc.sync.dma_start(out=outraw[c, :, :], in_=osb)

    nc.compile()
    return nc


def _prep_weights(kernel_w, rec_kernel, bias, A1_w, A2_w, v):
    b0, b1 = bias[0], bias[1]
    w = {}
    KzN = np.zeros((101, U), np.float32)
    KzN[:E] = -kernel_w[:, :U]
    KzN[100, :] = -40.0
    Kr = np.zeros((101, U), np.float32)
    Kr[:E] = kernel_w[:, U:2 * U]
    Kh = np.zeros((101, U), np.float32)
    Kh[:E] = kernel_w[:, 2 * U:]
    RzN = np.zeros((101, U), np.float32)
    RzN[:U] = -rec_kernel[:, :U]
    RzN[100, :] = -(b0[:U] + b1[:U])
    Rr = np.zeros((101, U), np.float32)
    Rr[:U] = rec_kernel[:, U:2 * U]
    Rr[100, :] = b0[U:2 * U] + b1[U:2 * U]
    Rh = np.zeros((101, U), np.float32)
    Rh[:U] = rec_kernel[:, 2 * U:]
    Rh[100, :] = b1[2 * U:]
    w["wKzN"], w["wKr"], w["wKh"] = KzN, Kr, Kh
    w["wRzN"], w["wRr"], w["wRh"] = RzN, Rr, Rh
    w = {k: val.astype(bf16) for k, val in w.items()}
    w["wb0h"] = b0[2 * U:][:, None].astype(np.float32)
    w["wones"] = np.ones((1, G * BC), bf16)
    w["wA1"] = A1_w.astype(bf16)
    w["wA2"] = A2_w.astype(bf16)
    w["wVr"] = np.broadcast_to(v[0][:, None], (U, U)).astype(bf16).copy()
    w["wI"] = np.eye(U, dtype=np.float32).astype(bf16)
    return w


def kernel(session_hidden, mask, kernel, rec_kernel, bias, A1_w, A2_w, v, _trace=False):
    session_hidden = np.asarray(session_hidden, np.float32)
    mask = np.asarray(mask, np.float32)
    kernel_w = np.asarray(kernel, np.float32)
    rec_kernel = np.asarray(rec_kernel, np.float32)
    bias = np.asarray(bias, np.float32)
    A1_w = np.asarray(A1_w, np.float32)
    A2_w = np.asarray(A2_w, np.float32)
    v = np.asarray(v, np.float32)

    if "nc" not in _CACHE:
        _CACHE["nc"] = _build()
    nc = _CACHE["nc"]

    w = _prep_weights(kernel_w, rec_kernel, bias, A1_w, A2_w, v)

    # xaug: [NB, 101, G, NCHUNK, BC] flattened; rows 0:100 = x^T, row 100 = 1-m
    x = session_hidden.reshape(NCORES, NCHUNK, BC, T, E)
    m = mask.reshape(NCORES, NCHUNK, BC, T)
    in_maps = []
    for k in range(NCORES):
        xa = np.zeros((NB, 101, G, NCHUNK, BC), np.float32)
        # x[k]: [NCHUNK, BC, T, E] -> [T, E, NCHUNK, BC] -> [NB, G, E, NCHUNK, BC]
        xt = x[k].transpose(2, 3, 0, 1).reshape(NB, G, E, NCHUNK, BC)
        xa[:, :E] = xt.transpose(0, 2, 1, 3, 4)
        mm_ = (1.0 - m[k]).transpose(2, 0, 1).reshape(NB, G, NCHUNK, BC)
        xa[:, 100] = mm_
        im = dict(w)
        im["xaug"] = xa.reshape(NB, 101, G * NCHUNK * BC).astype(bf16)
        in_maps.append(im)

    res = bass_utils.run_bass_kernel_spmd(nc, in_maps, core_ids=list(range(NCORES)),
                                          trace=_trace)
    _CACHE["last_res"] = res

    out_raw = np.zeros((B, U), np.float32)
    last = np.zeros((B, U), np.float32)
    for k in range(NCORES):
        r = res.results[k]
        for c in range(NCHUNK):
            sl = slice(k * PERCORE + c * BC, k * PERCORE + (c + 1) * BC)
            out_raw[sl] = np.asarray(r["outraw"][c]).T.astype(np.float32)
            last[sl] = np.asarray(r["lastout"][c]).T.astype(np.float32)

    # host correction for masked timesteps (device used last@A1 term for ALL t)
    lengths = mask.sum(1)
    sl_ = last @ A2_w
    c_ = last @ A1_w
    sig = lambda a: 1.0 / (1.0 + np.exp(-a))
    a_corr = (sig(sl_ + c_) - sig(sl_)) @ v[0]
    out = out_raw - (T - lengths)[:, None] * a_corr[:, None] * last
    return out.astype(np.float32)
